# revision 1
# baseline (speedup 1.0000x reference)
"""PointNet feature interpolation (3-NN inverse-distance) Trainium2 kernel.

Problem (per batch b of 8, one NeuronCore each):
  xyz1:    [3, N=8192]   source point coords
  xyz2:    [3, S=2048]   query point coords
  points1: [D=256, N]    source features
  out:     [D, S]        interpolated features

Device algorithm per core (v3):
  1. negdist[s, n] = 2*x2_s.x1_n - |x2_s|^2 - |x1_n|^2  (= -squared distance)
     as ONE K=30 bf16 matmul; fp32 inputs are pre-split on the host into
     (hi, lo, qlo) bf16 triples with all significant cross products as
     separate contraction rows (error ~1e-6 = fp32-reference level).
  2. PSUM -> SBUF copy casts to bf16; a pairwise tensor-max tree (DVE 2x
     bf16 mode) folds 8192 -> 512 "block maxima" (block j = points
     {j + 512m}).  vector.max/max_index give the top-8 blocks per row.
  3. Blocks of [x,y,z,|x1|^2] are gathered by indirect DMA from a host-built
     table; candidate distances for 8x16 points/row are recomputed exactly
     in fp32; vector.max gives the top-3 values; a batched equality-match
     against the candidate array recovers global indices.
  4. weights w_k = (1/(d_k+1e-8)) / sum; features gathered by indirect DMA
     from p1t = points1.T; weighted sum; PE-transpose to [D, S]; DMA out.
"""

import numpy as np
import ml_dtypes

B, N, S, D = 8, 8192, 2048, 256
P = 128
NCHUNK = S // P      # 16 query-row chunks per core
NT = 512             # matmul moving free dim (one PSUM bank)
NNT = N // NT        # 16
K = 30               # contraction rows of the distance matmul
NBLK = 512           # block-maxima per row (block = 16 points stride 512)
BPTS = N // NBLK     # 16 points per block
NB = 8               # candidate blocks kept per row
NCAND = NB * BPTS    # 128 candidate points per row

_COMPILED = None


def _build_bass(reps=1, abl=()):
    import concourse.bass as bass
    import concourse.mybir as mybir
    import concourse.tile as tile
    from concourse import bacc
    from concourse.masks import make_identity

    f32 = mybir.dt.float32
    bf16 = mybir.dt.bfloat16
    u32 = mybir.dt.uint32
    Alu = mybir.AluOpType
    X = mybir.AxisListType.X
    XY = mybir.AxisListType.XY

    nc = bacc.Bacc(None)
    x2m = nc.dram_tensor("x2m", [K, S], bf16, kind="ExternalInput")
    x1m = nc.dram_tensor("x1m", [K, N], bf16, kind="ExternalInput")
    p1t = nc.dram_tensor("p1t", [N, D], f32, kind="ExternalInput")
    xblk = nc.dram_tensor("xblk", [NBLK, BPTS * 4], f32, kind="ExternalInput")
    x2n = nc.dram_tensor("x2n", [P, NCHUNK, 4], f32, kind="ExternalInput")
    outT = nc.dram_tensor("outT", [D, S], f32, kind="ExternalOutput")

    with tile.TileContext(nc) as tc:
        with (
            tc.tile_pool(name="const", bufs=1) as cpool,
            tc.tile_pool(name="negb", bufs=2) as nbpool,
            tc.tile_pool(name="tree", bufs=2) as tpool,
            tc.tile_pool(name="mm", bufs=2, space="PSUM") as mmpool,
            tc.tile_pool(name="small", bufs=4) as spool,
            tc.tile_pool(name="gat", bufs=4) as gpool,
            tc.tile_pool(name="persist", bufs=1) as ppool,
        ):
            x2s = cpool.tile([K, S], bf16)
            nc.sync.dma_start(x2s[:], x2m[:])
            x1s = cpool.tile([K, N], bf16)
            nc.sync.dma_start(x1s[:], x1m[:])
            x2n_sb = cpool.tile([P, NCHUNK, 4], f32)
            nc.sync.dma_start(x2n_sb[:], x2n[:])
            ident = cpool.tile([P, P], f32)
            make_identity(nc, ident[:])
            # iotam[p, k, m] = 512*m + 1  (the +1 biases gmap so that a
            # failed equality match, max 0, decodes to index -1, never hit)
            iotam = cpool.tile([P, NB, BPTS], u32)
            nc.gpsimd.iota(iotam[:], pattern=[[0, NB], [NBLK, BPTS]],
                           base=1, channel_multiplier=0)

            for _rep in range(reps):
                # persistent per-rep state
                bi_all = ppool.tile([P, NCHUNK, NB], u32, tag="bi")
                gxb = ppool.tile([P, NCHUNK, NB, BPTS * 4], f32, tag="gxb")
                negdc = ppool.tile([P, NCHUNK, NB, BPTS], f32, tag="negdc")
                cv8 = ppool.tile([P, NCHUNK, 8], f32, tag="cv8")

                # ---- pass 1: distances, block maxima, top-8 blocks ----
                for ci in range(NCHUNK):
                    negb = nbpool.tile([P, N], bf16)
                    for q in range(4):
                        ps = mmpool.tile([P, 2048], f32, tag="mm")
                        if "nomm" in abl:
                            nc.vector.memset(ps[:, 0:8], 1.0)
                        else:
                            for j in range(4):
                                nt = q * 4 + j
                                nc.tensor.matmul(
                                    ps[:, j * NT:(j + 1) * NT],
                                    lhsT=x2s[:, ci * P:(ci + 1) * P],
                                    rhs=x1s[:, nt * NT:(nt + 1) * NT],
                                    start=True,
                                    stop=True,
                                )
                        if "halfcopy" in abl and q >= 2:
                            pass
                        elif "nocopy" in abl and q >= 1:
                            pass
                        else:
                            nc.scalar.copy(negb[:, q * 2048:(q + 1) * 2048], ps[:])

                    # pairwise-max tree 8192 -> 512 (bf16 2x mode)
                    bm = tpool.tile([P, NBLK], bf16, tag="bm")
                    if "notree" in abl:
                        nc.vector.tensor_copy(bm[:], negb[:, 0:NBLK])
                    else:
                        tsc = tpool.tile([P, 4096], bf16, tag="tree")
                        nc.vector.tensor_tensor(
                            out=tsc[:], in0=negb[:, 0:4096], in1=negb[:, 4096:8192],
                            op=Alu.max)
                        nc.vector.tensor_tensor(
                            out=tsc[:, 0:2048], in0=tsc[:, 0:2048],
                            in1=tsc[:, 2048:4096], op=Alu.max)
                        nc.vector.tensor_tensor(
                            out=tsc[:, 0:1024], in0=tsc[:, 0:1024],
                            in1=tsc[:, 1024:2048], op=Alu.max)
                        nc.vector.tensor_tensor(
                            out=bm[:], in0=tsc[:, 0:512], in1=tsc[:, 512:1024],
                            op=Alu.max)

                    bv8 = spool.tile([P, 8], bf16)
                    if "nosearch" in abl:
                        nc.vector.memset(bv8[:], -1.0)
                        nc.vector.memset(bi_all[:, ci, :], 5)
                    else:
                        nc.vector.max(out=bv8[:], in_=bm[:])
                        nc.vector.max_index(out=bi_all[:, ci, :], in_max=bv8[:],
                                            in_values=bm[:])

                    # gather candidate blocks [x,y,z,n1] x 16 points
                    for k in range(NB):
                        if "noblkgather" in abl:
                            nc.sync.dma_start(gxb[:, ci, k, :],
                                              xblk[(k*61) % 384:(k*61) % 384 + P, :])
                        else:
                            nc.gpsimd.indirect_dma_start(
                                out=gxb[:, ci, k, :],
                                out_offset=None,
                                in_=xblk[:],
                                in_offset=bass.IndirectOffsetOnAxis(
                                    ap=bi_all[:, ci, k:k + 1], axis=0),
                            )

                # ---- candidate distances (batched fp32, broadcast APs) ----
                gv = gxb[:].rearrange("p c k (m f) -> p c k m f", f=4)
                sh = [P, NCHUNK, NB, BPTS]

                def x2c(c):
                    return (x2n_sb[:, :, c:c + 1].unsqueeze(3)
                            .to_broadcast(sh))

                tmp = ppool.tile(sh, f32, tag="tmpA")
                tmp2 = ppool.tile(sh, f32, tag="tmpB")
                if "noarith" in abl:
                    nc.vector.memset(negdc[:, 0, 0, 0:8], -1.0)
                if "noarith" not in abl:
                    nc.vector.tensor_tensor(out=tmp[:], in0=gv[:, :, :, :, 0],
                                            in1=x2c(0), op=Alu.mult)
                    nc.vector.tensor_tensor(out=tmp2[:], in0=gv[:, :, :, :, 1],
                                            in1=x2c(1), op=Alu.mult)
                    nc.vector.tensor_tensor(out=tmp[:], in0=tmp[:], in1=tmp2[:],
                                            op=Alu.add)
                    nc.vector.tensor_tensor(out=tmp2[:], in0=gv[:, :, :, :, 2],
                                            in1=x2c(2), op=Alu.mult)
                    nc.vector.tensor_tensor(out=tmp[:], in0=tmp[:], in1=tmp2[:],
                                            op=Alu.add)
                    # negdc = 2*e - n2 - n1
                    nc.vector.tensor_scalar(out=tmp[:], in0=tmp[:], scalar1=2.0,
                                            scalar2=None, op0=Alu.mult)
                    nc.vector.tensor_tensor(out=tmp[:], in0=tmp[:], in1=x2c(3),
                                            op=Alu.subtract)
                    nc.vector.tensor_tensor(out=negdc[:], in0=tmp[:],
                                            in1=gv[:, :, :, :, 3],
                                            op=Alu.subtract)

                # top-8 candidate values per chunk (need top-3)
                for ci in range(NCHUNK):
                    nc.vector.max(out=cv8[:, ci, :], in_=negdc[:, ci])

                # ---- batched index decode: match values, take gmap ----
                gmap = ppool.tile([P, NCHUNK, NB, BPTS], u32, tag="gmap")
                nc.gpsimd.tensor_tensor(
                    out=gmap[:],
                    in0=bi_all[:].unsqueeze(3).to_broadcast(sh),
                    in1=iotam[:].unsqueeze(1).to_broadcast(sh),
                    op=Alu.add)
                shq = [P, NCHUNK, 3, NCAND]
                ndflat = negdc[:].rearrange("p c k m -> p c (k m)")
                gmflat = gmap[:].rearrange("p c k m -> p c (k m)")
                eqt = ppool.tile(shq, u32, tag="eqt")
                if "nodecode" in abl:
                    nc.vector.memset(eqt[:, 0, 0, 0:8], 3)
                n3 = ppool.tile([P, NCHUNK, 3], u32, tag="n3")
                if "nodecode" in abl:
                    nc.vector.memset(n3[:], 8)
                else:
                    nc.vector.tensor_tensor(
                        out=eqt[:],
                        in0=ndflat.unsqueeze(2).to_broadcast(shq),
                        in1=cv8[:, :, 0:3].unsqueeze(3).to_broadcast(shq),
                        op=Alu.is_equal)
                    nc.vector.tensor_tensor(
                        out=eqt[:], in0=eqt[:],
                        in1=gmflat.unsqueeze(2).to_broadcast(shq),
                        op=Alu.mult)
                    nc.vector.tensor_reduce(out=n3[:], in_=eqt[:], axis=X,
                                            op=Alu.max)
                nc.vector.tensor_scalar(out=n3[:], in0=n3[:], scalar1=1,
                                        scalar2=None, op0=Alu.subtract)

                # ---- batched weights ----
                d3 = ppool.tile([P, NCHUNK, 3], f32, tag="d3")
                nc.vector.tensor_scalar(out=d3[:], in0=cv8[:, :, 0:3],
                                        scalar1=-1.0, scalar2=1e-8,
                                        op0=Alu.mult, op1=Alu.add)
                nc.vector.reciprocal(d3[:], d3[:])
                rsum = ppool.tile([P, NCHUNK], f32, tag="rsum")
                nc.vector.tensor_reduce(out=rsum[:], in_=d3[:], axis=X,
                                        op=Alu.add)
                nc.vector.reciprocal(rsum[:], rsum[:])
                w3 = ppool.tile([P, NCHUNK, 3], f32, tag="w3")
                nc.vector.tensor_tensor(
                    out=w3[:], in0=d3[:],
                    in1=rsum[:].unsqueeze(2).to_broadcast([P, NCHUNK, 3]),
                    op=Alu.mult)

                # ---- pass 2: feature gather, interpolate, transpose ----
                for ci in range(NCHUNK):
                    g = gpool.tile([P, 3, D], f32, tag="g")
                    for k in range(3):
                        if "nofgather" in abl:
                            nc.sync.dma_start(g[:, k, :], p1t[k * P:(k + 1) * P, :])
                        else:
                            nc.gpsimd.indirect_dma_start(
                                out=g[:, k, :],
                                out_offset=None,
                                in_=p1t[:],
                                in_offset=bass.IndirectOffsetOnAxis(
                                    ap=n3[:, ci, k:k + 1], axis=0),
                            )
                    acc = gpool.tile([P, D], f32, tag="acc")
                    if "nointerp" in abl:
                        nc.vector.memset(acc[:, 0:8], 0.5)
                    else:
                        nc.scalar.mul(acc[:], g[:, 0, :], w3[:, ci, 0:1])
                        for k in (1, 2):
                            gm = gpool.tile([P, D], f32, tag="gm")
                            nc.scalar.mul(gm[:], g[:, k, :], w3[:, ci, k:k + 1])
                            nc.vector.tensor_add(acc[:], acc[:], gm[:])

                    for dh in range(2):
                        pt = mmpool.tile([P, P], f32, tag="mm")
                        nc.tensor.transpose(
                            pt[:], acc[:, dh * P:(dh + 1) * P], ident[:])
                        ot = gpool.tile([P, P], f32, tag="ot")
                        nc.scalar.copy(ot[:], pt[:])
                        nc.sync.dma_start(
                            outT[dh * P:(dh + 1) * P, ci * P:(ci + 1) * P],
                            ot[:])

    nc.finalize()
    return nc


def _split3(x):
    """Split fp64 array into 3 bf16 terms h+l+q ~ x (residual ~2^-27|x|)."""
    bf = ml_dtypes.bfloat16
    h = x.astype(bf)
    r = x - h.astype(np.float64)
    l = r.astype(bf)
    r2 = r - l.astype(np.float64)
    q = r2.astype(bf)
    return h, l, q


def _host_matrices(xyz2b, xyz1b):
    """Build the K=30 bf16 contraction matrices for one batch.

    negdist[s, n] = sum_k X2[k, s] * X1[k, n]
                  = 2 * x2_s . x1_n - |x2_s|^2 - |x1_n|^2
    """
    bf = ml_dtypes.bfloat16
    x2 = xyz2b.astype(np.float64)   # [3, S]
    x1 = xyz1b.astype(np.float64)   # [3, N]
    n2 = (x2 * x2).sum(axis=0)      # [S]
    n1 = (x1 * x1).sum(axis=0)      # [N]

    Srows, Nrows = [], []
    for c in range(3):
        h2, l2, q2 = _split3(x2[c])
        h1, l1, q1 = _split3(x1[c])
        th2 = (2.0 * h2.astype(np.float64)).astype(bf)
        tl2 = (2.0 * l2.astype(np.float64)).astype(bf)
        tq2 = (2.0 * q2.astype(np.float64)).astype(bf)
        # products kept: hh hl lh hq qh ll lq ql   (qq dropped)
        for a, b_ in ((th2, h1), (th2, l1), (tl2, h1), (th2, q1),
                      (tq2, h1), (tl2, l1), (tl2, q1), (tq2, l1)):
            Srows.append(a)
            Nrows.append(b_)
    ones_s = np.ones(x2.shape[1], dtype=bf)
    ones_n = np.ones(x1.shape[1], dtype=bf)
    for t in _split3(-n2):
        Srows.append(t)
        Nrows.append(ones_n)
    for t in _split3(-n1):
        Srows.append(ones_s)
        Nrows.append(t)
    X2 = np.stack([np.asarray(r, dtype=bf) for r in Srows])   # [30, S]
    X1 = np.stack([np.asarray(r, dtype=bf) for r in Nrows])   # [30, N]
    return X2, X1, n2.astype(np.float32), n1.astype(np.float32)


def _prep_inputs(xyz1, xyz2, points1):
    xyz1 = np.asarray(xyz1, dtype=np.float32)
    xyz2 = np.asarray(xyz2, dtype=np.float32)
    points1 = np.asarray(points1, dtype=np.float32)
    in_maps = []
    for b in range(B):
        X2, X1, n2, n1 = _host_matrices(xyz2[b], xyz1[b])
        p1tb = np.ascontiguousarray(points1[b].T)  # [N, D]
        # block table: row j holds points {j + 512*m}, each [x, y, z, n1]
        xb = np.empty((NBLK, BPTS, 4), dtype=np.float32)
        pts = xyz1[b].T.reshape(BPTS, NBLK, 3)     # [m, j, 3]
        xb[:, :, 0:3] = pts.transpose(1, 0, 2)
        xb[:, :, 3] = n1.reshape(BPTS, NBLK).T
        # per-query [x, y, z, n2], laid out [p, chunk, 4]
        xq = np.empty((P, NCHUNK, 4), dtype=np.float32)
        q = xyz2[b].T.reshape(NCHUNK, P, 3)        # [chunk, p, 3]
        xq[:, :, 0:3] = q.transpose(1, 0, 2)
        xq[:, :, 3] = n2.reshape(NCHUNK, P).T
        in_maps.append({
            "x2m": X2, "x1m": X1, "p1t": p1tb,
            "xblk": xb.reshape(NBLK, BPTS * 4), "x2n": xq,
        })
    return in_maps


def _get_compiled():
    global _COMPILED
    if _COMPILED is None:
        _COMPILED = _build_bass()
    return _COMPILED


def kernel(xyz1, xyz2, points1):
    from concourse.bass_utils import run_bass_kernel_spmd

    nc = _get_compiled()
    in_maps = _prep_inputs(xyz1, xyz2, points1)
    res = run_bass_kernel_spmd(nc, in_maps, core_ids=list(range(B)))
    return np.stack([r["outT"] for r in res.results]).astype(np.float32)


if __name__ == "__main__":
    rng = np.random.default_rng(0)
    xyz1 = rng.standard_normal((B, 3, N), dtype=np.float32)
    xyz2 = rng.standard_normal((B, 3, S), dtype=np.float32)
    p1 = rng.standard_normal((B, D, N), dtype=np.float32)
    out = kernel(xyz1, xyz2, p1)
    print("out", out.shape, out.dtype)
    import test as T
    gt = T.np_reference_fp64(xyz1, xyz2, p1)
    diff = out.astype(np.float64) - gt.astype(np.float64)
    print("L2rel vs fp64:", np.linalg.norm(diff) / np.linalg.norm(gt))
    colmax = np.abs(diff).max(axis=1)
    print("rows > 0.01:", int((colmax > 0.01).sum()), "/", colmax.size)



# revision 19
# speedup vs baseline: 2.1926x; 2.1926x over previous
"""PointNet feature interpolation (3-NN inverse-distance) Trainium2 kernel.

Problem (per batch b of 8, one NeuronCore each):
  xyz1:    [3, N=8192]   source point coords
  xyz2:    [3, S=2048]   query point coords
  points1: [D=256, N]    source features
  out:     [D, S]        interpolated features

Device algorithm per core (v3):
  1. negdist[s, n] = 2*x2_s.x1_n - |x2_s|^2 - |x1_n|^2  (= -squared distance)
     as ONE K=30 bf16 matmul; fp32 inputs are pre-split on the host into
     (hi, lo, qlo) bf16 triples with all significant cross products as
     separate contraction rows (error ~1e-6 = fp32-reference level).
  2. PSUM -> SBUF copy casts to bf16; a pairwise tensor-max tree (DVE 2x
     bf16 mode) folds 8192 -> 512 "block maxima" (block j = points
     {j + 512m}).  vector.max/max_index give the top-8 blocks per row.
  3. Blocks of [x,y,z,|x1|^2] are gathered by indirect DMA from a host-built
     table; candidate distances for 8x16 points/row are recomputed exactly
     in fp32; vector.max gives the top-3 values; a batched equality-match
     against the candidate array recovers global indices.
  4. weights w_k = (1/(d_k+1e-8)) / sum; features gathered by indirect DMA
     from p1t = points1.T; weighted sum; PE-transpose to [D, S]; DMA out.
"""

import numpy as np
import ml_dtypes

B, N, S, D = 8, 8192, 2048, 256
P = 128
NCHUNK = S // P      # 16 query-row chunks per core
NT = 512             # matmul moving free dim (one PSUM bank)
NNT = N // NT        # 16
K = 30               # contraction rows of the distance matmul
NBLK = 512           # block-maxima per row (block = 16 points stride 512)
BPTS = N // NBLK     # 16 points per block
NB = 6               # candidate blocks kept per row
NCAND = NB * BPTS    # 128 candidate points per row

_COMPILED = None


def _build_bass(reps=1, abl=()):
    import concourse.bass as bass
    import concourse.mybir as mybir
    import concourse.tile as tile
    from concourse import bacc

    f32 = mybir.dt.float32
    bf16 = mybir.dt.bfloat16
    u32 = mybir.dt.uint32
    Alu = mybir.AluOpType
    X = mybir.AxisListType.X
    XY = mybir.AxisListType.XY

    nc = bacc.Bacc(None)
    x2m = nc.dram_tensor("x2m", [K, S], bf16, kind="ExternalInput")
    x1m = nc.dram_tensor("x1m", [K, N], bf16, kind="ExternalInput")
    p1t = nc.dram_tensor("p1t", [N, D], bf16, kind="ExternalInput")
    xblk = nc.dram_tensor("xblk", [NBLK, BPTS * 4], f32, kind="ExternalInput")
    x2n = nc.dram_tensor("x2n", [P, NCHUNK, 4], f32, kind="ExternalInput")
    outS = nc.dram_tensor("outS", [S, D], f32, kind="ExternalOutput")

    with tile.TileContext(nc) as tc:
        with (
            tc.tile_pool(name="const", bufs=1) as cpool,
            tc.tile_pool(name="negb", bufs=2) as nbpool,
            tc.tile_pool(name="tree", bufs=2) as tpool,
            tc.tile_pool(name="mm", bufs=2, space="PSUM") as mmpool,
            tc.tile_pool(name="small", bufs=4) as spool,
            tc.tile_pool(name="gat", bufs=4) as gpool,
            tc.tile_pool(name="persist", bufs=1) as ppool,
        ):
            x2s = cpool.tile([K, S], bf16)
            nc.sync.dma_start(x2s[:], x2m[:])
            x1s = cpool.tile([K, N], bf16)
            nc.sync.dma_start(x1s[:], x1m[:])
            x2n_sb = cpool.tile([P, NCHUNK, 4], f32)
            nc.sync.dma_start(x2n_sb[:], x2n[:])
            # iotam[p, k, m] = 512*m + 1  (the +1 biases gmap so that a
            # failed equality match, max 0, decodes to index -1, never hit)
            iotam = cpool.tile([P, NB, BPTS], u32)
            nc.gpsimd.iota(iotam[:], pattern=[[0, NB], [NBLK, BPTS]],
                           base=1, channel_multiplier=0)

            for _rep in range(reps):
                # persistent per-rep state
                bi_all = ppool.tile([P, NCHUNK, 8], u32, tag="bi")
                gxb = ppool.tile([P, NCHUNK, NB, BPTS * 4], f32, tag="gxb")
                negdc = ppool.tile([P, NCHUNK, NB, BPTS], f32, tag="negdc")
                cv8 = ppool.tile([P, NCHUNK, 8], f32, tag="cv8")

                # ---- pass 1: distances, block maxima, top-8 blocks ----
                for ci in range(NCHUNK):
                    negb = nbpool.tile([P, N], bf16)
                    for q in range(4):
                        ps = mmpool.tile([P, 2048], f32, tag="mm")
                        if "nomm" in abl:
                            nc.vector.memset(ps[:, 0:8], 1.0)
                        else:
                            for j in range(4):
                                nt = q * 4 + j
                                nc.tensor.matmul(
                                    ps[:, j * NT:(j + 1) * NT],
                                    lhsT=x2s[:, ci * P:(ci + 1) * P],
                                    rhs=x1s[:, nt * NT:(nt + 1) * NT],
                                    start=True,
                                    stop=True,
                                )
                        if "halfcopy" in abl and q >= 2:
                            pass
                        elif "nocopy" in abl and q >= 1:
                            pass
                        else:
                            nc.scalar.copy(negb[:, q * 2048:(q + 1) * 2048], ps[:])

                    # pairwise-max tree 8192 -> 512 (bf16 2x mode)
                    bm = tpool.tile([P, NBLK], bf16, tag="bm")
                    if "notree" in abl:
                        nc.vector.tensor_copy(bm[:], negb[:, 0:NBLK])
                    else:
                        tsc = tpool.tile([P, 4096], bf16, tag="tree")
                        nc.vector.tensor_tensor(
                            out=tsc[:], in0=negb[:, 0:4096], in1=negb[:, 4096:8192],
                            op=Alu.max)
                        nc.vector.tensor_tensor(
                            out=tsc[:, 0:2048], in0=tsc[:, 0:2048],
                            in1=tsc[:, 2048:4096], op=Alu.max)
                        nc.vector.tensor_tensor(
                            out=tsc[:, 0:1024], in0=tsc[:, 0:1024],
                            in1=tsc[:, 1024:2048], op=Alu.max)
                        nc.vector.tensor_tensor(
                            out=bm[:], in0=tsc[:, 0:512], in1=tsc[:, 512:1024],
                            op=Alu.max)

                    bv8 = spool.tile([P, 8], bf16)
                    if "nosearch" in abl:
                        nc.vector.memset(bv8[:], -1.0)
                        nc.vector.memset(bi_all[:, ci, :], 5)
                    else:
                        nc.vector.max(out=bv8[:], in_=bm[:])
                        nc.vector.max_index(out=bi_all[:, ci, :], in_max=bv8[:],
                                            in_values=bm[:])

                    # gather candidate blocks [x,y,z,n1] x 16 points
                    for k in range(NB):
                        if "noblkgather" in abl:
                            nc.sync.dma_start(gxb[:, ci, k, :],
                                              xblk[(k*61) % 384:(k*61) % 384 + P, :])
                        else:
                            nc.gpsimd.indirect_dma_start(
                                out=gxb[:, ci, k, :],
                                out_offset=None,
                                in_=xblk[:],
                                in_offset=bass.IndirectOffsetOnAxis(
                                    ap=bi_all[:, ci, k:k + 1], axis=0),
                            )

                # ---- candidate distances (batched fp32, broadcast APs) ----
                gv = gxb[:].rearrange("p c k (m f) -> p c k m f", f=4)
                sh = [P, NCHUNK, NB, BPTS]

                def x2c(c):
                    return (x2n_sb[:, :, c:c + 1].unsqueeze(3)
                            .to_broadcast(sh))

                tmp = ppool.tile(sh, f32, tag="tmpA")
                tmp2 = ppool.tile(sh, f32, tag="tmpB")
                if "noarith" in abl:
                    nc.vector.memset(negdc[:, 0, 0, 0:8], -1.0)
                if "noarith" not in abl:
                    nc.vector.tensor_tensor(out=tmp[:], in0=gv[:, :, :, :, 0],
                                            in1=x2c(0), op=Alu.mult)
                    nc.vector.tensor_tensor(out=tmp2[:], in0=gv[:, :, :, :, 1],
                                            in1=x2c(1), op=Alu.mult)
                    nc.vector.tensor_tensor(out=tmp[:], in0=tmp[:], in1=tmp2[:],
                                            op=Alu.add)
                    nc.vector.tensor_tensor(out=tmp2[:], in0=gv[:, :, :, :, 2],
                                            in1=x2c(2), op=Alu.mult)
                    nc.vector.tensor_tensor(out=tmp[:], in0=tmp[:], in1=tmp2[:],
                                            op=Alu.add)
                    # negdc = 2*e - n2 - n1
                    nc.vector.tensor_scalar(out=tmp[:], in0=tmp[:], scalar1=2.0,
                                            scalar2=None, op0=Alu.mult)
                    nc.vector.tensor_tensor(out=tmp[:], in0=tmp[:], in1=x2c(3),
                                            op=Alu.subtract)
                    nc.vector.tensor_tensor(out=negdc[:], in0=tmp[:],
                                            in1=gv[:, :, :, :, 3],
                                            op=Alu.subtract)

                # top-8 candidate values per chunk (need top-3)
                for ci in range(NCHUNK):
                    nc.vector.max(out=cv8[:, ci, :], in_=negdc[:, ci])

                # ---- batched index decode: match values, take gmap ----
                gmap = ppool.tile([P, NCHUNK, NB, BPTS], u32, tag="gmap")
                nc.gpsimd.tensor_tensor(
                    out=gmap[:],
                    in0=bi_all[:, :, 0:NB].unsqueeze(3).to_broadcast(sh),
                    in1=iotam[:].unsqueeze(1).to_broadcast(sh),
                    op=Alu.add)
                shq = [P, NCHUNK, 3, NCAND]
                ndflat = negdc[:].rearrange("p c k m -> p c (k m)")
                gmflat = gmap[:].rearrange("p c k m -> p c (k m)")
                eqt = ppool.tile(shq, u32, tag="eqt")
                if "nodecode" in abl:
                    nc.vector.memset(eqt[:, 0, 0, 0:8], 3)
                n3 = ppool.tile([P, NCHUNK, 3], u32, tag="n3")
                if "nodecode" in abl:
                    nc.vector.memset(n3[:], 8)
                else:
                    nc.vector.tensor_tensor(
                        out=eqt[:],
                        in0=ndflat.unsqueeze(2).to_broadcast(shq),
                        in1=cv8[:, :, 0:3].unsqueeze(3).to_broadcast(shq),
                        op=Alu.is_equal)
                    nc.vector.tensor_tensor(
                        out=eqt[:], in0=eqt[:],
                        in1=gmflat.unsqueeze(2).to_broadcast(shq),
                        op=Alu.mult)
                    nc.vector.tensor_reduce(out=n3[:], in_=eqt[:], axis=X,
                                            op=Alu.max)
                nc.vector.tensor_scalar(out=n3[:], in0=n3[:], scalar1=1,
                                        scalar2=None, op0=Alu.subtract)

                # ---- batched weights ----
                d3 = ppool.tile([P, NCHUNK, 3], f32, tag="d3")
                nc.vector.tensor_scalar(out=d3[:], in0=cv8[:, :, 0:3],
                                        scalar1=-1.0, scalar2=1e-8,
                                        op0=Alu.mult, op1=Alu.add)
                nc.vector.reciprocal(d3[:], d3[:])
                rsum = ppool.tile([P, NCHUNK], f32, tag="rsum")
                nc.vector.tensor_reduce(out=rsum[:], in_=d3[:], axis=X,
                                        op=Alu.add)
                nc.vector.reciprocal(rsum[:], rsum[:])
                w3 = ppool.tile([P, NCHUNK, 3], f32, tag="w3")
                nc.vector.tensor_tensor(
                    out=w3[:], in0=d3[:],
                    in1=rsum[:].unsqueeze(2).to_broadcast([P, NCHUNK, 3]),
                    op=Alu.mult)

                # ---- pass 2: feature gather, interpolate, transpose ----
                for ci in range(NCHUNK):
                    g = gpool.tile([P, 3, D], bf16, tag="g")
                    for k in range(3):
                        if "nofgather" in abl:
                            nc.sync.dma_start(g[:, k, :], p1t[k * P:(k + 1) * P, :])
                        else:
                            nc.gpsimd.indirect_dma_start(
                                out=g[:, k, :],
                                out_offset=None,
                                in_=p1t[:],
                                in_offset=bass.IndirectOffsetOnAxis(
                                    ap=n3[:, ci, k:k + 1], axis=0),
                            )
                    acc = gpool.tile([P, D], f32, tag="acc")
                    if "nointerp" in abl:
                        nc.vector.memset(acc[:, 0:8], 0.5)
                    else:
                        nc.scalar.mul(acc[:], g[:, 0, :], w3[:, ci, 0:1])
                        for k in (1, 2):
                            gm = gpool.tile([P, D], f32, tag="gm")
                            nc.scalar.mul(gm[:], g[:, k, :], w3[:, ci, k:k + 1])
                            nc.vector.tensor_add(acc[:], acc[:], gm[:])

                    nc.sync.dma_start(outS[ci * P:(ci + 1) * P, :], acc[:])

    nc.finalize()
    return nc


def _split3(x):
    """Split fp64 array into 3 bf16 terms h+l+q ~ x (residual ~2^-27|x|)."""
    bf = ml_dtypes.bfloat16
    h = x.astype(bf)
    r = x - h.astype(np.float64)
    l = r.astype(bf)
    r2 = r - l.astype(np.float64)
    q = r2.astype(bf)
    return h, l, q


def _host_matrices(xyz2b, xyz1b):
    """Build the K=30 bf16 contraction matrices for one batch.

    negdist[s, n] = sum_k X2[k, s] * X1[k, n]
                  = 2 * x2_s . x1_n - |x2_s|^2 - |x1_n|^2
    """
    bf = ml_dtypes.bfloat16
    x2 = xyz2b.astype(np.float64)   # [3, S]
    x1 = xyz1b.astype(np.float64)   # [3, N]
    n2 = (x2 * x2).sum(axis=0)      # [S]
    n1 = (x1 * x1).sum(axis=0)      # [N]

    Srows, Nrows = [], []
    for c in range(3):
        h2, l2, q2 = _split3(x2[c])
        h1, l1, q1 = _split3(x1[c])
        th2 = (2.0 * h2.astype(np.float64)).astype(bf)
        tl2 = (2.0 * l2.astype(np.float64)).astype(bf)
        tq2 = (2.0 * q2.astype(np.float64)).astype(bf)
        # products kept: hh hl lh hq qh ll lq ql   (qq dropped)
        for a, b_ in ((th2, h1), (th2, l1), (tl2, h1), (th2, q1),
                      (tq2, h1), (tl2, l1), (tl2, q1), (tq2, l1)):
            Srows.append(a)
            Nrows.append(b_)
    ones_s = np.ones(x2.shape[1], dtype=bf)
    ones_n = np.ones(x1.shape[1], dtype=bf)
    for t in _split3(-n2):
        Srows.append(t)
        Nrows.append(ones_n)
    for t in _split3(-n1):
        Srows.append(ones_s)
        Nrows.append(t)
    X2 = np.stack([np.asarray(r, dtype=bf) for r in Srows])   # [30, S]
    X1 = np.stack([np.asarray(r, dtype=bf) for r in Nrows])   # [30, N]
    return X2, X1, n2.astype(np.float32), n1.astype(np.float32)


def _prep_inputs(xyz1, xyz2, points1):
    xyz1 = np.asarray(xyz1, dtype=np.float32)
    xyz2 = np.asarray(xyz2, dtype=np.float32)
    points1 = np.asarray(points1, dtype=np.float32)
    in_maps = []
    for b in range(B):
        X2, X1, n2, n1 = _host_matrices(xyz2[b], xyz1[b])
        p1tb = np.ascontiguousarray(points1[b].T).astype(
            ml_dtypes.bfloat16)  # [N, D] bf16
        # block table: row j holds points {j + 512*m}, each [x, y, z, n1]
        xb = np.empty((NBLK, BPTS, 4), dtype=np.float32)
        pts = xyz1[b].T.reshape(BPTS, NBLK, 3)     # [m, j, 3]
        xb[:, :, 0:3] = pts.transpose(1, 0, 2)
        xb[:, :, 3] = n1.reshape(BPTS, NBLK).T
        # per-query [x, y, z, n2], laid out [p, chunk, 4]
        xq = np.empty((P, NCHUNK, 4), dtype=np.float32)
        q = xyz2[b].T.reshape(NCHUNK, P, 3)        # [chunk, p, 3]
        xq[:, :, 0:3] = q.transpose(1, 0, 2)
        xq[:, :, 3] = n2.reshape(NCHUNK, P).T
        in_maps.append({
            "x2m": X2, "x1m": X1, "p1t": p1tb,
            "xblk": xb.reshape(NBLK, BPTS * 4), "x2n": xq,
        })
    return in_maps


def _get_compiled():
    global _COMPILED
    if _COMPILED is None:
        _COMPILED = _build_bass()
    return _COMPILED


def kernel(xyz1, xyz2, points1):
    from concourse.bass_utils import run_bass_kernel_spmd

    nc = _get_compiled()
    in_maps = _prep_inputs(xyz1, xyz2, points1)
    res = run_bass_kernel_spmd(nc, in_maps, core_ids=list(range(B)))
    out = np.stack([r["outS"] for r in res.results])     # [B, S, D]
    return np.ascontiguousarray(out.transpose(0, 2, 1)).astype(np.float32)


if __name__ == "__main__":
    rng = np.random.default_rng(0)
    xyz1 = rng.standard_normal((B, 3, N), dtype=np.float32)
    xyz2 = rng.standard_normal((B, 3, S), dtype=np.float32)
    p1 = rng.standard_normal((B, D, N), dtype=np.float32)
    out = kernel(xyz1, xyz2, p1)
    print("out", out.shape, out.dtype)
